# revision 12
# baseline (speedup 1.0000x reference)
"""Causal self-attention (B=2, T=2048, C=1024, H=16) on 8 TRN2 NeuronCores.

Sharding: data parallel over batch (2) x tensor parallel over heads (4 groups
of 4 heads). Each core computes qkv + attention for its 4 heads of one batch,
normalized attention outputs are AllGathered (chunked over t, overlapped with
attention) within each batch group of 4 cores, and each core then computes a
256-column slice of the output projection. The host concatenates the column
slices (pure gather, no reduction).
"""
import numpy as np
import ml_dtypes

import concourse.bass as bass
import concourse.tile as tile
from concourse import bacc, mybir
from concourse.bass_utils import run_bass_kernel_spmd

BF16 = ml_dtypes.bfloat16

B, T, C, H, D = 2, 2048, 1024, 16, 64
NCORES = 8
HPC = 4              # heads per core
FQK = 2 * HPC * D    # 512 rows of q+k per core
FV = HPC * D         # 256 rows of v per core
CT = C // 128        # 8 contraction tiles
TC5 = T // 512       # 4 t-chunks of 512
SB = T // 128        # 16 s-blocks of 128
SCALE = 1.0 / 8.0    # 1/sqrt(D)

_CACHE = {}


def _build_kernel():
    nc = bacc.Bacc("TRN2", target_bir_lowering=False, debug=False,
                   num_devices=NCORES)
    dt = mybir.dt
    f32, bf16 = dt.float32, dt.bfloat16

    xT = nc.dram_tensor("xT", [C, T], bf16, kind="ExternalInput").ap()
    wqkT = nc.dram_tensor("wqkT", [C, FQK], bf16, kind="ExternalInput").ap()
    wvT = nc.dram_tensor("wvT", [C, FV], bf16, kind="ExternalInput").ap()
    wpT = nc.dram_tensor("wpT", [C, FV], bf16, kind="ExternalInput").ap()
    bqk = nc.dram_tensor("bqk", [FQK, 1], f32, kind="ExternalInput").ap()
    bv = nc.dram_tensor("bv", [1, FV], bf16, kind="ExternalInput").ap()
    bp = nc.dram_tensor("bp", [1, FV], bf16, kind="ExternalInput").ap()
    maskb = nc.dram_tensor("maskb", [128, 896], bf16, kind="ExternalInput").ap()
    onesr = nc.dram_tensor("onesr", [1, 64], mybir.dt.float32r,
                           kind="ExternalInput").ap()
    out = nc.dram_tensor("out", [T, FV], f32, kind="ExternalOutput").ap()

    with tile.TileContext(nc) as tc:
        with (
            tc.tile_pool(name="persist", bufs=1) as pp,
            tc.tile_pool(name="work", bufs=4) as wp,
            tc.tile_pool(name="attT", bufs=6) as ap_pool,
            tc.tile_pool(name="outsb", bufs=3) as op,
            tc.tile_pool(name="ps_qk", bufs=2, space="PSUM") as ps_qk,
            tc.tile_pool(name="ps_y", bufs=2, space="PSUM") as ps_y,
            tc.tile_pool(name="ps_bc", bufs=1, space="PSUM") as ps_bc,
            tc.tile_pool(name="ps_mm", bufs=1, space="PSUM") as ps_mm,
            tc.tile_pool(name="dram", bufs=1, space="DRAM") as dram,
        ):
            # ---- load constants / inputs to SBUF (split for pipelining;
            #      first t-chunk of x^T and the qk weights land first) ----
            wqk_s = pp.tile([128, CT, FQK], bf16, tag="wqk")
            xT_s = pp.tile([128, CT, T], bf16, tag="xT")
            wv_s = pp.tile([128, CT, FV], bf16, tag="wv")
            xTr = xT.rearrange("(n p) t -> p n t", p=128)
            wqkr = wqkT.rearrange("(n p) f -> p n f", p=128)
            wvr = wvT.rearrange("(n p) f -> p n f", p=128)
            bqk_s = pp.tile([128, 4], f32, tag="bqk")
            nc.sync.dma_start(bqk_s[:], bqk.rearrange("(n p) o -> p (n o)", p=128))
            for ci in range(CT):
                nc.sync.dma_start(wqk_s[:, ci, :], wqkr[:, ci, :])
                nc.sync.dma_start(xT_s[:, ci, 0:512], xTr[:, ci, 0:512])
            for ci in range(CT):
                nc.sync.dma_start(wv_s[:, ci, :], wvr[:, ci, :])
            bv_s = pp.tile([1, FV], bf16, tag="bv")
            nc.sync.dma_start(bv_s[:], bv[:])
            mask_s = pp.tile([128, 896], bf16, tag="mask")
            nc.sync.dma_start(mask_s[:], maskb[:])
            for t5x in range(1, TC5):
                for ci in range(CT):
                    nc.sync.dma_start(xT_s[:, ci, t5x * 512:(t5x + 1) * 512],
                                      xTr[:, ci, t5x * 512:(t5x + 1) * 512])
            wp_s = pp.tile([128, CT, FV], bf16, tag="wp")
            nc.sync.dma_start(wp_s[:], wpT.rearrange("(n p) f -> p n f", p=128))
            bp_s = pp.tile([1, FV], bf16, tag="bp")
            nc.sync.dma_start(bp_s[:], bp[:])
            ones16 = pp.tile([1, 128], bf16, tag="ones16")
            nc.vector.memset(ones16[:], 1.0)
            ones32 = pp.tile([1, 64], mybir.dt.float32r, tag="ones32")
            nc.sync.dma_start(ones32[:], onesr[:])
            # broadcast bias rows to all 128 partitions once (K=1 matmuls)
            bv_bc = pp.tile([128, FV], f32, tag="bv_bc")
            bp_bc = pp.tile([128, FV], f32, tag="bp_bc")
            for row, bc_t in ((bv_s, bv_bc), (bp_s, bp_bc)):
                psb = ps_mm.tile([128, FV], f32, tag="mm", name="ps_bias")
                nc.tensor.matmul(psb[:], ones16[0:1, :], row[0:1, :],
                                 start=True, stop=True)
                nc.vector.tensor_copy(bc_t[:], psb[:])

            # ---- qkT / v chunk emitters (interleaved into attention loop) ----
            qkT_s = [pp.tile([128, T], bf16, tag=f"qkT{fc}", name=f"qkT{fc}")
                     for fc in range(4)]
            vaug = pp.tile([128, SB, HPC * 65], bf16, tag="vaug")
            nc.vector.memset(vaug[:], 1.0)

            def qkT_chunk(t5):
                for fc in range(4):
                    ps = ps_qk.tile([128, 512], f32, tag="qk", name="ps_qkv")
                    for ci in range(CT):
                        nc.tensor.matmul(
                            ps[:],
                            wqk_s[:, ci, fc * 128:(fc + 1) * 128],
                            xT_s[:, ci, t5 * 512:(t5 + 1) * 512],
                            start=(ci == 0), stop=(ci == CT - 1),
                        )
                    nc.vector.tensor_scalar_add(
                        qkT_s[fc][:, t5 * 512:(t5 + 1) * 512], ps[:],
                        bqk_s[:, fc:fc + 1],
                    )

            def v_chunk(t5):
                for tb in range(4 * t5, 4 * t5 + 4):
                    ps = ps_mm.tile([128, FV], f32, tag="mm", name="ps_v")
                    for ci in range(CT):
                        nc.tensor.matmul(
                            ps[:],
                            xT_s[:, ci, tb * 128:(tb + 1) * 128],
                            wv_s[:, ci, :],
                            start=(ci == 0), stop=(ci == CT - 1),
                        )
                    dst = vaug[:, tb, :].rearrange("p (h x) -> p h x", h=HPC)[:, :, 0:64]
                    src = ps[:].rearrange("p (h x) -> p h x", h=HPC)
                    bias = bv_bc[:].rearrange("p (h x) -> p h x", h=HPC)
                    nc.vector.scalar_tensor_tensor(
                        dst, src, 1.0, bias,
                        op0=mybir.AluOpType.mult, op1=mybir.AluOpType.add,
                    )

            # ---- attention, t-chunk major; AG + proj pipelined behind it ----
            ag_in, ag_out, yf = {}, {}, {}

            def proj_chunk(t5):
                yft = yf[t5]
                for tq in range(4):
                    tb = t5 * 4 + tq
                    pso = ps_mm.tile([128, FV], f32, tag="mm", name="ps_o")
                    for ci in range(CT):
                        par, cc = ci % 2, ci // 2
                        nc.tensor.matmul(
                            pso[:],
                            yft[:, par, cc, tq * 128:(tq + 1) * 128],
                            wp_s[:, ci, :],
                            start=(ci == 0), stop=(ci == CT - 1),
                        )
                    osb = op.tile([128, FV], f32, tag="osb", name="osb")
                    nc.vector.tensor_add(osb[:], pso[:], bp_bc[:])
                    nc.sync.dma_start(out[tb * 128:(tb + 1) * 128, :], osb[:])

            attD = {}
            for offz in (128, 256, 384):
                attD[offz] = pp.tile([128, 1024], bf16, tag=f"attD{offz}",
                                     name=f"attD{offz}")
                zv = attD[offz][:].rearrange("p (g x) -> p g x", g=2)
                nc.vector.memset(zv[:, :, 0:offz], 0.0)
            qkT_chunk(0)
            v_chunk(0)
            for t5 in range(TC5):
                if t5 + 1 < TC5:
                    qkT_chunk(t5 + 1)
                    v_chunk(t5 + 1)
                live = 4 * (t5 + 1)
                for pr in range(2):
                    ag_in[(t5, pr)] = dram.tile([128, 512], bf16,
                                                tag=f"agin{t5}_{pr}",
                                                name=f"agin{t5}_{pr}")
                    ag_out[(t5, pr)] = dram.tile([512, 512], bf16,
                                                 tag=f"agout{t5}_{pr}",
                                                 name=f"agout{t5}_{pr}")
                yf[t5] = pp.tile([128, 2, CT // 2, 512], bf16, tag="yf", bufs=2,
                                 name=f"yf{t5}")
                for pair in range(2):
                    q_fc, k_fc = pair, 2 + pair
                    ypsA = ps_y.tile([65, 512], f32, tag="y", name="ypsA")
                    ypsB = ps_y.tile([65, 512], f32, tag="y", name="ypsB")
                    for sb in range(live):
                        ps = ps_qk.tile([128, 1024], f32, tag="qk", name="ps_s")
                        for hh in range(2):
                            lo, hi = 64 * hh, 64 * (hh + 1)
                            nc.tensor.matmul(
                                ps[:, hh * 512:(hh + 1) * 512],
                                qkT_s[k_fc][lo:hi, sb * 128:(sb + 1) * 128],
                                qkT_s[q_fc][lo:hi, t5 * 512:(t5 + 1) * 512],
                                start=True, stop=True,
                            )
                        off = sb * 128 - t5 * 512
                        if off > 0:
                            a = attD[off]  # zero cols [0, off) pre-set, kept
                        else:
                            a = ap_pool.tile([128, 1024], bf16, tag="attT",
                                             name="attT")
                        av = a[:].rearrange("p (g x) -> p g x", g=2)
                        pv = ps[:].rearrange("p (g x) -> p g x", g=2)
                        if off > 0:
                            nc.scalar.activation(
                                av[:, :, off:512], pv[:, :, off:512],
                                mybir.ActivationFunctionType.Exp, scale=SCALE,
                            )
                        else:
                            nc.scalar.activation(
                                a[:], ps[:],
                                mybir.ActivationFunctionType.Exp, scale=SCALE,
                            )
                        if off >= 0:
                            # boundary 128 cols get the partial causal mask
                            msl = mask_s[:, 384:512]
                            nc.vector.tensor_mul(av[:, 0, off:off + 128],
                                                 av[:, 0, off:off + 128], msl)
                            nc.vector.tensor_mul(av[:, 1, off:off + 128],
                                                 av[:, 1, off:off + 128], msl)
                        for hh, yps in ((0, ypsA), (1, ypsB)):
                            h = pair * 2 + hh
                            nc.tensor.matmul(
                                yps[:],
                                vaug[:, sb, h * 65:(h + 1) * 65],
                                a[:, hh * 512:(hh + 1) * 512],
                                start=(sb == 0), stop=(sb == live - 1),
                            )
                    # normalize: y / denom (denom = row 64 via ones column)
                    for hh, yps in ((0, ypsA), (1, ypsB)):
                        h = pair * 2 + hh
                        den = wp.tile([1, 512], mybir.dt.float32r, tag="den",
                                      name="den")
                        nc.vector.tensor_copy(den[:], yps[64:65, :])
                        bc = ps_bc.tile([64, 512], f32, tag="bc", name="bc")
                        nc.tensor.matmul(bc[:], ones32[0:1, :], den[:],
                                         start=True, stop=True)
                        r = wp.tile([64, 512], f32, tag="recip", name="recip")
                        nc.vector.reciprocal_approx_fast(r[:], bc[:])
                        yn = wp.tile([64, 512], bf16, tag="yn", name="yn")
                        nc.vector.tensor_mul(yn[:], yps[0:64, :], r[:])
                        nc.sync.dma_start(
                            ag_in[(t5, pair)][hh * 64:(hh + 1) * 64, :], yn[:])
                    nc.gpsimd.collective_compute(
                        "AllGather", mybir.AluOpType.bypass,
                        replica_groups=[[0, 1, 2, 3], [4, 5, 6, 7]],
                        ins=[ag_in[(t5, pair)][:].opt()],
                        outs=[ag_out[(t5, pair)][:].opt()],
                    )
                    nc.gpsimd.dma_start(
                        yf[t5][:, pair, :, :],
                        ag_out[(t5, pair)][:].rearrange("(n p) t -> p n t", p=128))
                if t5 == 2:
                    proj_chunk(0)
            proj_chunk(1)
            proj_chunk(2)
            proj_chunk(3)

    nc.compile()
    return nc


def _shard_inputs(x, w_attn, b_attn, w_proj, b_proj):
    mask = np.zeros((128, 896), dtype=BF16)
    for p in range(128):
        mask[p, 384 + p:] = 1.0

    in_maps = []
    for core in range(NCORES):
        b, hg = core // 4, core % 4
        r0 = hg * HPC * D          # first q/k/v row offset within each 1024
        r1 = r0 + HPC * D
        wqk = np.concatenate([w_attn[r0:r1, :], w_attn[C + r0:C + r1, :]], 0)
        in_maps.append({
            "xT": np.ascontiguousarray(x[b].T).astype(BF16),
            "wqkT": np.ascontiguousarray(wqk.T).astype(BF16),
            "wvT": np.ascontiguousarray(w_attn[2 * C + r0:2 * C + r1, :].T).astype(BF16),
            "wpT": np.ascontiguousarray(w_proj[r0:r1, :].T).astype(BF16),
            "bqk": np.concatenate([b_attn[r0:r1], b_attn[C + r0:C + r1]])
                     .reshape(FQK, 1).astype(np.float32),
            "bv": b_attn[2 * C + r0:2 * C + r1].reshape(1, FV).astype(BF16),
            "bp": b_proj[r0:r1].reshape(1, FV).astype(BF16),
            "maskb": mask,
            "onesr": np.ones((1, 64), dtype=np.float32),
        })
    return in_maps


def kernel(x, w_attn, b_attn, w_proj, b_proj, _trace=False, _trace_kwargs=None):
    x = np.asarray(x, dtype=np.float32)
    w_attn = np.asarray(w_attn, dtype=np.float32)
    b_attn = np.asarray(b_attn, dtype=np.float32)
    w_proj = np.asarray(w_proj, dtype=np.float32)
    b_proj = np.asarray(b_proj, dtype=np.float32)

    if "nc" not in _CACHE:
        _CACHE["nc"] = _build_kernel()
    nc = _CACHE["nc"]

    in_maps = _shard_inputs(x, w_attn, b_attn, w_proj, b_proj)
    res = run_bass_kernel_spmd(nc, in_maps, core_ids=list(range(NCORES)),
                               trace=_trace, **(_trace_kwargs or {}))
    _CACHE["last_result"] = res

    out = np.empty((B, T, C), dtype=np.float32)
    for core in range(NCORES):
        b, hg = core // 4, core % 4
        out[b, :, hg * FV:(hg + 1) * FV] = res.results[core]["out"]
    return out


# revision 13
# speedup vs baseline: 1.0604x; 1.0604x over previous
"""Causal self-attention (B=2, T=2048, C=1024, H=16) on 8 TRN2 NeuronCores.

Sharding: data parallel over batch (2) x tensor parallel over heads (4 groups
of 4 heads). Each core computes qkv + attention for its 4 heads of one batch,
normalized attention outputs are AllGathered (chunked over t, overlapped with
attention) within each batch group of 4 cores, and each core then computes a
256-column slice of the output projection. The host concatenates the column
slices (pure gather, no reduction).
"""
import numpy as np
import ml_dtypes

import concourse.bass as bass
import concourse.tile as tile
from concourse import bacc, mybir
from concourse.bass_utils import run_bass_kernel_spmd

BF16 = ml_dtypes.bfloat16

B, T, C, H, D = 2, 2048, 1024, 16, 64
NCORES = 8
HPC = 4              # heads per core
FQK = 2 * HPC * D    # 512 rows of q+k per core
FV = HPC * D         # 256 rows of v per core
CT = C // 128        # 8 contraction tiles
TC5 = T // 512       # 4 t-chunks of 512
SB = T // 128        # 16 s-blocks of 128
SCALE = 1.0 / 8.0    # 1/sqrt(D)

_CACHE = {}


def _build_kernel():
    nc = bacc.Bacc("TRN2", target_bir_lowering=False, debug=False,
                   num_devices=NCORES)
    dt = mybir.dt
    f32, bf16 = dt.float32, dt.bfloat16

    xT = nc.dram_tensor("xT", [C, T], bf16, kind="ExternalInput").ap()
    wqkT = nc.dram_tensor("wqkT", [C, FQK], bf16, kind="ExternalInput").ap()
    wvT = nc.dram_tensor("wvT", [C, FV], bf16, kind="ExternalInput").ap()
    wpT = nc.dram_tensor("wpT", [C, FV], bf16, kind="ExternalInput").ap()
    bqk = nc.dram_tensor("bqk", [FQK, 1], f32, kind="ExternalInput").ap()
    bv = nc.dram_tensor("bv", [1, FV], bf16, kind="ExternalInput").ap()
    bp = nc.dram_tensor("bp", [1, FV], bf16, kind="ExternalInput").ap()
    maskb = nc.dram_tensor("maskb", [128, 896], bf16, kind="ExternalInput").ap()
    onesr = nc.dram_tensor("onesr", [1, 64], mybir.dt.float32r,
                           kind="ExternalInput").ap()
    out = nc.dram_tensor("out", [T, FV], f32, kind="ExternalOutput").ap()

    with tile.TileContext(nc) as tc:
        with (
            tc.tile_pool(name="persist", bufs=1) as pp,
            tc.tile_pool(name="work", bufs=4) as wp,
            tc.tile_pool(name="attT", bufs=6) as ap_pool,
            tc.tile_pool(name="outsb", bufs=3) as op,
            tc.tile_pool(name="ps_qk", bufs=2, space="PSUM") as ps_qk,
            tc.tile_pool(name="ps_y", bufs=2, space="PSUM") as ps_y,
            tc.tile_pool(name="ps_bc", bufs=1, space="PSUM") as ps_bc,
            tc.tile_pool(name="ps_mm", bufs=1, space="PSUM") as ps_mm,
            tc.tile_pool(name="dram", bufs=1, space="DRAM") as dram,
        ):
            # ---- load constants / inputs to SBUF (split for pipelining;
            #      first t-chunk of x^T and the qk weights land first) ----
            wqk_s = pp.tile([128, CT, FQK], bf16, tag="wqk")
            xT_s = pp.tile([128, CT, T], bf16, tag="xT")
            wv_s = pp.tile([128, CT, FV], bf16, tag="wv")
            xTr = xT.rearrange("(n p) t -> p n t", p=128)
            wqkr = wqkT.rearrange("(n p) f -> p n f", p=128)
            wvr = wvT.rearrange("(n p) f -> p n f", p=128)
            bqk_s = pp.tile([128, 4], f32, tag="bqk")
            nc.sync.dma_start(bqk_s[:], bqk.rearrange("(n p) o -> p (n o)", p=128))
            for ci in range(CT):
                nc.sync.dma_start(wqk_s[:, ci, :], wqkr[:, ci, :])
                nc.sync.dma_start(xT_s[:, ci, 0:512], xTr[:, ci, 0:512])
            for ci in range(CT):
                nc.sync.dma_start(wv_s[:, ci, :], wvr[:, ci, :])
            bv_s = pp.tile([1, FV], bf16, tag="bv")
            nc.sync.dma_start(bv_s[:], bv[:])
            mask_s = pp.tile([128, 896], bf16, tag="mask")
            nc.sync.dma_start(mask_s[:], maskb[:])
            for t5x in range(1, TC5):
                for ci in range(CT):
                    nc.sync.dma_start(xT_s[:, ci, t5x * 512:(t5x + 1) * 512],
                                      xTr[:, ci, t5x * 512:(t5x + 1) * 512])
            wp_s = pp.tile([128, CT, FV], bf16, tag="wp")
            nc.sync.dma_start(wp_s[:], wpT.rearrange("(n p) f -> p n f", p=128))
            bp_s = pp.tile([1, FV], bf16, tag="bp")
            nc.sync.dma_start(bp_s[:], bp[:])
            ones16 = pp.tile([1, 128], bf16, tag="ones16")
            nc.vector.memset(ones16[:], 1.0)
            ones32 = pp.tile([1, 64], mybir.dt.float32r, tag="ones32")
            nc.sync.dma_start(ones32[:], onesr[:])
            # broadcast bias rows to all 128 partitions once (K=1 matmuls)
            bv_bc = pp.tile([128, FV], f32, tag="bv_bc")
            bp_bc = pp.tile([128, FV], f32, tag="bp_bc")
            for row, bc_t in ((bv_s, bv_bc), (bp_s, bp_bc)):
                psb = ps_mm.tile([128, FV], f32, tag="mm", name="ps_bias")
                nc.tensor.matmul(psb[:], ones16[0:1, :], row[0:1, :],
                                 start=True, stop=True)
                nc.vector.tensor_copy(bc_t[:], psb[:])

            # ---- qkT / v chunk emitters (interleaved into attention loop) ----
            qkT_s = [pp.tile([128, T], bf16, tag=f"qkT{fc}", name=f"qkT{fc}")
                     for fc in range(4)]
            vaug = pp.tile([128, SB, HPC * 65], bf16, tag="vaug")
            nc.vector.memset(vaug[:], 1.0)

            def qkT_chunk(t5):
                for fc in range(4):
                    ps = ps_qk.tile([128, 512], f32, tag="qk", name="ps_qkv")
                    for ci in range(CT):
                        nc.tensor.matmul(
                            ps[:],
                            wqk_s[:, ci, fc * 128:(fc + 1) * 128],
                            xT_s[:, ci, t5 * 512:(t5 + 1) * 512],
                            start=(ci == 0), stop=(ci == CT - 1),
                        )
                    nc.vector.tensor_scalar_add(
                        qkT_s[fc][:, t5 * 512:(t5 + 1) * 512], ps[:],
                        bqk_s[:, fc:fc + 1],
                    )

            def v_chunk(t5):
                for tb in range(4 * t5, 4 * t5 + 4):
                    ps = ps_mm.tile([128, FV], f32, tag="mm", name="ps_v")
                    for ci in range(CT):
                        nc.tensor.matmul(
                            ps[:],
                            xT_s[:, ci, tb * 128:(tb + 1) * 128],
                            wv_s[:, ci, :],
                            start=(ci == 0), stop=(ci == CT - 1),
                        )
                    dst = vaug[:, tb, :].rearrange("p (h x) -> p h x", h=HPC)[:, :, 0:64]
                    src = ps[:].rearrange("p (h x) -> p h x", h=HPC)
                    bias = bv_bc[:].rearrange("p (h x) -> p h x", h=HPC)
                    nc.vector.scalar_tensor_tensor(
                        dst, src, 1.0, bias,
                        op0=mybir.AluOpType.mult, op1=mybir.AluOpType.add,
                    )

            # ---- attention, t-chunk major; AG + proj pipelined behind it ----
            ag_in, ag_out, yf = {}, {}, {}

            def proj_chunk(t5):
                yft = yf[t5]
                for tq in range(4):
                    tb = t5 * 4 + tq
                    pso = ps_mm.tile([128, FV], f32, tag="mm", name="ps_o")
                    for ci in range(CT):
                        par, cc = ci % 2, ci // 2
                        nc.tensor.matmul(
                            pso[:],
                            yft[:, par, cc, tq * 128:(tq + 1) * 128],
                            wp_s[:, ci, :],
                            start=(ci == 0), stop=(ci == CT - 1),
                        )
                    osb = op.tile([128, FV], f32, tag="osb", name="osb")
                    nc.vector.tensor_add(osb[:], pso[:], bp_bc[:])
                    nc.sync.dma_start(out[tb * 128:(tb + 1) * 128, :], osb[:])

            qkT_chunk(0)
            v_chunk(0)
            for t5 in range(TC5):
                if t5 + 1 < TC5:
                    qkT_chunk(t5 + 1)
                    v_chunk(t5 + 1)
                live = 4 * (t5 + 1)
                for pr in range(2):
                    ag_in[(t5, pr)] = dram.tile([128, 512], bf16,
                                                tag=f"agin{t5}_{pr}",
                                                name=f"agin{t5}_{pr}")
                    ag_out[(t5, pr)] = dram.tile([512, 512], bf16,
                                                 tag=f"agout{t5}_{pr}",
                                                 name=f"agout{t5}_{pr}")
                yf[t5] = pp.tile([128, 2, CT // 2, 512], bf16, tag="yf", bufs=2,
                                 name=f"yf{t5}")
                for pair in range(2):
                    q_fc, k_fc = pair, 2 + pair
                    ypsA = ps_y.tile([65, 512], f32, tag="y", name="ypsA")
                    ypsB = ps_y.tile([65, 512], f32, tag="y", name="ypsB")
                    for sb in range(live):
                        ps = ps_qk.tile([128, 1024], f32, tag="qk", name="ps_s")
                        for hh in range(2):
                            lo, hi = 64 * hh, 64 * (hh + 1)
                            nc.tensor.matmul(
                                ps[:, hh * 512:(hh + 1) * 512],
                                qkT_s[k_fc][lo:hi, sb * 128:(sb + 1) * 128],
                                qkT_s[q_fc][lo:hi, t5 * 512:(t5 + 1) * 512],
                                start=True, stop=True,
                            )
                        a = ap_pool.tile([128, 1024], bf16, tag="attT",
                                         name="attT")
                        off = sb * 128 - t5 * 512
                        av = a[:].rearrange("p (g x) -> p g x", g=2)
                        pv = ps[:].rearrange("p (g x) -> p g x", g=2)
                        if off > 0:
                            # cols [0, off) are fully masked: zero instead of exp
                            nc.vector.memset(av[:, :, 0:off], 0.0)
                            nc.scalar.activation(
                                av[:, :, off:512], pv[:, :, off:512],
                                mybir.ActivationFunctionType.Exp, scale=SCALE,
                            )
                        else:
                            nc.scalar.activation(
                                a[:], ps[:],
                                mybir.ActivationFunctionType.Exp, scale=SCALE,
                            )
                        if off >= 0:
                            # boundary 128 cols get the partial causal mask
                            msl = mask_s[:, 384:512]
                            nc.vector.tensor_mul(av[:, 0, off:off + 128],
                                                 av[:, 0, off:off + 128], msl)
                            nc.vector.tensor_mul(av[:, 1, off:off + 128],
                                                 av[:, 1, off:off + 128], msl)
                        for hh, yps in ((0, ypsA), (1, ypsB)):
                            h = pair * 2 + hh
                            nc.tensor.matmul(
                                yps[:],
                                vaug[:, sb, h * 65:(h + 1) * 65],
                                a[:, hh * 512:(hh + 1) * 512],
                                start=(sb == 0), stop=(sb == live - 1),
                            )
                    # normalize: y / denom (denom = row 64 via ones column)
                    for hh, yps in ((0, ypsA), (1, ypsB)):
                        h = pair * 2 + hh
                        den = wp.tile([1, 512], mybir.dt.float32r, tag="den",
                                      name="den")
                        nc.vector.tensor_copy(den[:], yps[64:65, :])
                        bc = ps_bc.tile([64, 512], f32, tag="bc", name="bc")
                        nc.tensor.matmul(bc[:], ones32[0:1, :], den[:],
                                         start=True, stop=True)
                        r = wp.tile([64, 512], f32, tag="recip", name="recip")
                        nc.vector.reciprocal_approx_fast(r[:], bc[:])
                        yn = wp.tile([64, 512], bf16, tag="yn", name="yn")
                        nc.vector.tensor_mul(yn[:], yps[0:64, :], r[:])
                        nc.sync.dma_start(
                            ag_in[(t5, pair)][hh * 64:(hh + 1) * 64, :], yn[:])
                    nc.gpsimd.collective_compute(
                        "AllGather", mybir.AluOpType.bypass,
                        replica_groups=[[0, 1, 2, 3], [4, 5, 6, 7]],
                        ins=[ag_in[(t5, pair)][:].opt()],
                        outs=[ag_out[(t5, pair)][:].opt()],
                    )
                    nc.gpsimd.dma_start(
                        yf[t5][:, pair, :, :],
                        ag_out[(t5, pair)][:].rearrange("(n p) t -> p n t", p=128))
                if t5 >= 2:
                    proj_chunk(t5 - 2)
            proj_chunk(2)
            proj_chunk(3)

    nc.compile()
    return nc


def _shard_inputs(x, w_attn, b_attn, w_proj, b_proj):
    mask = np.zeros((128, 896), dtype=BF16)
    for p in range(128):
        mask[p, 384 + p:] = 1.0

    in_maps = []
    for core in range(NCORES):
        b, hg = core // 4, core % 4
        r0 = hg * HPC * D          # first q/k/v row offset within each 1024
        r1 = r0 + HPC * D
        wqk = np.concatenate([w_attn[r0:r1, :], w_attn[C + r0:C + r1, :]], 0)
        in_maps.append({
            "xT": np.ascontiguousarray(x[b].T).astype(BF16),
            "wqkT": np.ascontiguousarray(wqk.T).astype(BF16),
            "wvT": np.ascontiguousarray(w_attn[2 * C + r0:2 * C + r1, :].T).astype(BF16),
            "wpT": np.ascontiguousarray(w_proj[r0:r1, :].T).astype(BF16),
            "bqk": np.concatenate([b_attn[r0:r1], b_attn[C + r0:C + r1]])
                     .reshape(FQK, 1).astype(np.float32),
            "bv": b_attn[2 * C + r0:2 * C + r1].reshape(1, FV).astype(BF16),
            "bp": b_proj[r0:r1].reshape(1, FV).astype(BF16),
            "maskb": mask,
            "onesr": np.ones((1, 64), dtype=np.float32),
        })
    return in_maps


def kernel(x, w_attn, b_attn, w_proj, b_proj, _trace=False, _trace_kwargs=None):
    x = np.asarray(x, dtype=np.float32)
    w_attn = np.asarray(w_attn, dtype=np.float32)
    b_attn = np.asarray(b_attn, dtype=np.float32)
    w_proj = np.asarray(w_proj, dtype=np.float32)
    b_proj = np.asarray(b_proj, dtype=np.float32)

    if "nc" not in _CACHE:
        _CACHE["nc"] = _build_kernel()
    nc = _CACHE["nc"]

    in_maps = _shard_inputs(x, w_attn, b_attn, w_proj, b_proj)
    res = run_bass_kernel_spmd(nc, in_maps, core_ids=list(range(NCORES)),
                               trace=_trace, **(_trace_kwargs or {}))
    _CACHE["last_result"] = res

    out = np.empty((B, T, C), dtype=np.float32)
    for core in range(NCORES):
        b, hg = core // 4, core % 4
        out[b, :, hg * FV:(hg + 1) * FV] = res.results[core]["out"]
    return out
